# revision 30
# baseline (speedup 1.0000x reference)
"""Causal self-attention Trainium2 Bass kernel.

Shapes (hardcoded): B=8, T=1024, C=768, NH=12, HS=64.
Sharding: data-parallel over batch — core b computes batch element b.

Per-core dataflow (all matmuls bf16 with fp32 PSUM accumulation):
  - Inputs are host-repacked so every DMA line is one contiguous
    segment per partition (128 descriptors per DMA): xTp [128,KT,T]
    split per k-tile, wqkp per head pair, wvp/wpp whole. Input DMAs are
    spread over both HWDGE rings (Sync + Scalar) so transfers overlap.
  - qkT [2,T] per pair = wqk tiles (stationary) x xT (moving); k-outer
    / nch-inner so each weight tile feeds 2 consecutive matmuls.
  - v token-major, assembled into v_aug [jt, head, 65] with a ones
    column so the PV matmul also emits softmax row-sums for free.
  - S^T blocks per head, COMPACTED: each j-tile's valid column range
    (512 - lo) is packed back-to-back in the PSUM tile so the exp
    activation covers only valid columns in a single op per group.
    Causality via block skipping plus an additive -1920 lower-
    triangular constant accumulated into diagonal blocks by a bf16
    matmul (id.T @ mtri); exp(0.125*(S-1920)) underflows to exact 0.
  - exp via ScalarE activation (scale=1/8) PSUM->SBUF into bf16 P^T,
    software-pipelined with PV groups as before.
  - y^T [65, i] = v_aug.T x P^T accumulated in PSUM; row 64 is the
    softmax denominator. Normalize: srow copy + reciprocal (DVE),
    gpsimd partition_broadcast, DVE multiply straight out of PSUM
    into bf16 yT (no intermediate copy).
  - Chunk-major schedule: all pairs' chunk 0 first (with pair hp+2's
    QK projection interleaved so its ScalarE copies stay ahead of the
    exp queue), then all pairs' chunk 1 with output-projection tiles
    it0..3 interleaved (their yT columns completed with chunk 0), then
    it4..7 at the end. Each out tile [128, 768] DMAs independently so
    the tail drain is one tile deep.
"""

import numpy as np

import concourse.bass as bass
import concourse.mybir as mybir
import concourse.tile as tile
from concourse import bacc
from concourse.bass_utils import run_bass_kernel_spmd

B, T, C = 8, 1024, 768
NH, HS = 12, 64
NCORES = 8
KT = C // 128            # 6 contraction tiles
NPAIR = NH // 2          # 6 head pairs; head-pair hp covers heads 2hp, 2hp+1
F32 = mybir.dt.float32
BF16 = mybir.dt.bfloat16

# compacted causal layout per chunk c: j-tile widths / offsets.
# Block placement keeps every matmul PSUM write within one 512-f32 bank.
_CHUNK = {}
for _c in (0, 1):
    _njt = 4 * (_c + 1)
    _w = [512 - max(0, (_jt - 4 * _c) * 128) for _jt in range(_njt)]
    _lo = [512 - _wi for _wi in _w]
    _base = [sum(_w[:_jt]) for _jt in range(_njt)]
    _CHUNK[_c] = (_njt, _w, _lo, _base, sum(_w))

_cache = {}


def _build_program(bias_attn: bool, bias_proj: bool):
    nc = bacc.Bacc("TRN2", target_bir_lowering=False, debug=False,
                   num_devices=NCORES)

    xkp = nc.dram_tensor("xkp", [KT, 128, T], BF16, kind="ExternalInput")
    wqk0h = nc.dram_tensor("wqk0h", [2, 128, 3 * 256], BF16,
                           kind="ExternalInput")
    wqkp = nc.dram_tensor("wqkp", [128, NPAIR, KT, 256], BF16,
                          kind="ExternalInput")
    wvh = nc.dram_tensor("wvh", [2, 128, 3 * C], BF16, kind="ExternalInput")
    wpp = nc.dram_tensor("wpp", [128, KT, C], BF16, kind="ExternalInput")
    if bias_attn:
        bqk_d = nc.dram_tensor("bqk", [2 * C], F32, kind="ExternalInput")
        bv_d = nc.dram_tensor("bv", [C], F32, kind="ExternalInput")
    if bias_proj:
        bp_d = nc.dram_tensor("bp", [C], F32, kind="ExternalInput")
    # bf16 output (host casts back to f32): halves the writeback DMA
    out = nc.dram_tensor("out", [T, C], BF16, kind="ExternalOutput")

    # Constants merged into one inline tensor (one DMA): [id|mtri|mtri].
    # mtri is the additive causal mask for diagonal S^T blocks, applied
    # as a bf16 accumulation matmul (identity.T @ [mtri|mtri] adds mtri
    # to both diagonal blocks of a group in one instruction): -1920
    # pre-scale = -240 post-scale -> exp underflows to exactly 0.
    import ml_dtypes
    _mtri = np.tril(np.ones((128, 128), dtype=np.float32), k=-1) * -1920.0
    const_np = np.concatenate(
        [np.eye(128, dtype=np.float32), _mtri, _mtri],
        axis=1).astype(ml_dtypes.bfloat16)
    const_d = nc.inline_tensor(const_np, "idmtri")

    with tile.TileContext(nc) as tc:
        with (
            tc.tile_pool(name="xpool", bufs=1) as xpool,
            tc.tile_pool(name="cpool", bufs=1) as cpool,
            tc.tile_pool(name="wvpool", bufs=1) as wvpool,
            tc.tile_pool(name="vpool", bufs=1) as vpool,
            tc.tile_pool(name="wqkpool", bufs=1) as wqkpool,
            tc.tile_pool(name="qkpool", bufs=NPAIR) as qkpool,
            tc.tile_pool(name="ptpool", bufs=4) as ptpool,
            tc.tile_pool(name="ytpool", bufs=1) as ytpool,
            tc.tile_pool(name="wppool", bufs=1) as wppool,
            tc.tile_pool(name="opool", bufs=3) as opool,
            tc.tile_pool(name="smpool", bufs=4) as smpool,
            tc.tile_pool(name="psA", bufs=3, space="PSUM") as psA,
            tc.tile_pool(name="psB", bufs=2, space="PSUM") as psB,
        ):
            # ---- constants / bias staging (Scalar ring: keeps the Sync
            # ring free for the startup-critical wqk0/xT transfers) ----
            const_s = cpool.tile([128, 384], BF16, tag="idmtri")
            nc.scalar.dma_start(const_s[:], const_d.ap())
            id_s = const_s[:, 0:128]
            mtri2_s = const_s[:, 128:384]
            if bias_attn:
                bqk_s = cpool.tile([128, 12], F32, tag="bqk")
                nc.scalar.dma_start(
                    bqk_s[:], bqk_d.ap().rearrange("(m p) -> p m", p=128))
                bv_row = cpool.tile([1, C], F32, tag="bvrow")
                nc.scalar.dma_start(bv_row[:], bv_d.ap().rearrange("c -> 1 c"))
                bv_bc = cpool.tile([128, C], F32, tag="bvbc")
                nc.gpsimd.partition_broadcast(bv_bc[:], bv_row[:])
            if bias_proj:
                bp_row = cpool.tile([1, C], F32, tag="bprow")
                nc.scalar.dma_start(bp_row[:], bp_d.ap().rearrange("c -> 1 c"))
                bp_bc = cpool.tile([128, C], F32, tag="bpbc")
                nc.gpsimd.partition_broadcast(bp_bc[:], bp_row[:])

            ones3 = cpool.tile([128, NH, 1], F32, tag="ones3")
            nc.vector.memset(ones3[:], 1.0)

            # ---- input DMAs: few, fat-lined (contiguous per-partition
            # source rows), need-ordered across both HWDGE rings so HBM
            # bandwidth goes to whatever the PE needs next.
            wqk_s = wqkpool.tile([128, NPAIR, KT, 256], BF16, tag="wqk")
            xT_s = xpool.tile([128, KT, T], BF16, tag="xT")
            wv_s = wvpool.tile([128, KT, C], BF16, tag="wv")
            wp_s = wppool.tile([128, KT, C], BF16, tag="wp")
            wqk0h_r = wqk0h.ap().rearrange("h p m -> p h m")
            nc.sync.dma_start(
                wqk_s[:, 0, 0:3].rearrange("p k m -> p (k m)"),
                wqk0h_r[:, 0])
            nc.scalar.dma_start(
                wqk_s[:, 0, 3:6].rearrange("p k m -> p (k m)"),
                wqk0h_r[:, 1])
            xkp_r = xkp.ap().rearrange("k p t -> p k t")
            for k in range(KT):
                eng = nc.sync if k % 2 == 0 else nc.scalar
                eng.dma_start(xT_s[:, k], xkp_r[:, k])
            wvh_r = wvh.ap().rearrange("h p m -> p h m")
            nc.sync.dma_start(
                wv_s[:, 0:3].rearrange("p k m -> p (k m)"), wvh_r[:, 0])
            nc.scalar.dma_start(
                wv_s[:, 3:6].rearrange("p k m -> p (k m)"), wvh_r[:, 1])
            for hp in range(1, NPAIR):
                eng = nc.sync if hp % 2 == 1 else nc.scalar
                eng.dma_start(wqk_s[:, hp], wqkp.ap()[:, hp])
            nc.scalar.dma_start(wp_s[:], wpp.ap())

            def emit_qkproj(hp):
                qk_t = qkpool.tile([128, 2, T], BF16, tag="qk",
                                   name=f"qk_{hp}")
                for part in range(2):  # 0 = q m-tile hp, 1 = k m-tile hp
                    ps = psA.tile([128, 1024], F32, tag="big",
                                  name=f"qkps_{hp}_{part}")
                    for k in range(KT):
                        for nch in range(2):
                            nc.tensor.matmul(
                                ps[:, nch * 512:(nch + 1) * 512],
                                wqk_s[:, hp, k, part * 128:part * 128 + 128],
                                xT_s[:, k, nch * 512:(nch + 1) * 512],
                                start=(k == 0), stop=(k == KT - 1),
                            )
                    # PSUM->SBUF on ScalarE: lands in the ACT queue ahead
                    # of downstream exps (chunk-0 phase has Scalar slack).
                    if bias_attn:
                        nc.scalar.add(qk_t[:, part, :], ps[:],
                                      bqk_s[:, part * 6 + hp:part * 6 + hp + 1])
                    else:
                        nc.scalar.copy(qk_t[:, part, :], ps[:])
                return qk_t

            # ---- V: token-major, assembled as v_aug[jt, head, 65] ----
            v_aug = vpool.tile([128, 8, NH, HS + 1], BF16, tag="vaug")

            def emit_vproj():
                for jt in range(8):
                    ps = psA.tile([128, 1024], F32, tag="big",
                                  name=f"vps_{jt}")
                    for k in range(KT):
                        for off, w in ((0, 512), (512, 256)):
                            nc.tensor.matmul(
                                ps[:, off:off + w],
                                xT_s[:, k, jt * 128:(jt + 1) * 128],
                                wv_s[:, k, off:off + w],
                                start=(k == 0), stop=(k == KT - 1),
                            )
                    dst = v_aug[:, jt, :, 0:HS]
                    src = ps[:, 0:C].rearrange("p (h d) -> p h d", d=HS)
                    if bias_attn:
                        nc.vector.tensor_add(
                            dst, src,
                            bv_bc[:].rearrange("p (h d) -> p h d", d=HS))
                    else:
                        nc.vector.tensor_copy(dst, src)
                    nc.vector.tensor_copy(v_aug[:, jt, :, HS:HS + 1],
                                          ones3[:])

            # ---- yT accumulator (written during attention) ----
            yT_s = ytpool.tile([128, KT, T], BF16, tag="yT")

            def emit_attn_chunk(hp, qk_t, c, pending):
                """Emit one (pair, chunk). The last PV group + normalize
                are NOT emitted here — they are returned as a closure and
                emitted inside the NEXT chunk (after its first exp), so
                the in-order PE queue never stalls on the chunk-boundary
                exp -> PV -> normalize chain."""
                njt, wj, lo, base, Wc = _CHUNK[c]
                ng = njt // 2
                pts = [ptpool.tile([128, Wc], BF16, tag="pt",
                                   name=f"pt_{hp}_{hl}_{c}")
                       for hl in range(2)]
                y_pss = [psB.tile([128, 512], F32, tag="y",
                                  name=f"yps_{hp}_{hl}_{c}")
                         for hl in range(2)]

                def emit_pv(g):
                    for u in range(2):
                        jt = 2 * g + u
                        for hl in range(2):
                            nc.tensor.matmul(
                                y_pss[hl][0:HS + 1, lo[jt]:512],
                                v_aug[:, jt, 2 * hp + hl, :],
                                pts[hl][:, base[jt]:base[jt] + wj[jt]],
                                start=(jt == 0),
                                stop=(jt == njt - 1),
                                skip_group_check=(jt > 0),
                            )

                # Software-pipelined emission: PV of group g-1 between
                # S^T groups so the in-order PE queue always has work
                # that does not wait on the ScalarE exp.
                for g in range(ng):
                    sts = [psA.tile([128, 1024], F32, tag="big",
                                    name=f"st_{hp}_{hl}_{c}_{g}")
                           for hl in range(2)]
                    gbase = base[2 * g]
                    gw = wj[2 * g] + wj[2 * g + 1]
                    for u in range(2):
                        jt = 2 * g + u
                        b_in = base[jt] - gbase
                        diag = jt >= 4 * c
                        for hl in range(2):
                            bp = 64 * hl
                            nc.tensor.matmul(
                                sts[hl][:, b_in:b_in + wj[jt]],
                                qk_t[bp:bp + 64, 1,
                                     jt * 128:(jt + 1) * 128],
                                qk_t[bp:bp + 64, 0,
                                     c * 512 + lo[jt]:(c + 1) * 512],
                                start=True, stop=not diag,
                            )
                        if diag:
                            # accumulate -1920 on the j>i triangle; the
                            # diag block is the first 128 compacted cols
                            for hl in range(2):
                                nc.tensor.matmul(
                                    sts[hl][:, b_in:b_in + 128],
                                    id_s, mtri2_s[:, 0:128],
                                    start=False, stop=True,
                                    skip_group_check=True,
                                )
                    for hl in range(2):
                        nc.scalar.activation(
                            pts[hl][:, gbase:gbase + gw],
                            sts[hl][:, 0:gw],
                            mybir.ActivationFunctionType.Exp,
                            scale=0.125,
                        )
                    if g == 0 and pending is not None:
                        pending()
                    if g >= 1:
                        emit_pv(g - 1)

                def finish():
                    emit_pv(ng - 1)
                    # normalize: yT = y / sums, multiplying straight out
                    # of PSUM; batched by op type so the in-order Vector
                    # queue is not blocked by the gpsimd broadcast.
                    # copy from PSUM (shifts partition 64 -> 0; HW
                    # gpsimd broadcast reads partition 0 only) and recip
                    # in place (custom-DVE needs base partition 0)
                    srows = []
                    for hl in range(2):
                        srow = smpool.tile([1, 512], F32, tag="srow",
                                           name=f"srow_{hp}_{hl}_{c}")
                        nc.vector.tensor_copy(srow[:],
                                              y_pss[hl][HS:HS + 1, :])
                        nc.vector.reciprocal_approx_fast(srow[:], srow[:])
                        srows.append(srow)
                    sbcs = []
                    for hl in range(2):
                        sbc = smpool.tile([64, 512], F32, tag="sbc",
                                          name=f"sbc_{hp}_{hl}_{c}")
                        nc.gpsimd.partition_broadcast(sbc[:], srows[hl][:])
                        sbcs.append(sbc)
                    for hl in range(2):
                        nc.vector.tensor_mul(
                            yT_s[64 * hl:64 * hl + 64, hp,
                                 c * 512:(c + 1) * 512],
                            y_pss[hl][0:HS, :], sbcs[hl][:])
                return finish

            # ---- projection tile: out[it] = yT.T x w_proj (+ b_proj) ----
            def _proj_mms(ps, it, ks):
                for k in ks:
                    for off, w in ((0, 512), (512, 256)):
                        nc.tensor.matmul(
                            ps[:, off:off + w],
                            yT_s[:, k, it * 128:(it + 1) * 128],
                            wp_s[:, k, off:off + w],
                            start=(k == 0), stop=(k == KT - 1),
                        )

            def _proj_out(ps, it, dma_eng, copy_eng):
                ot = opool.tile([128, C], BF16, tag="ot", name=f"ot_{it}")
                if bias_proj:
                    nc.vector.tensor_add(ot[:], ps[:, 0:C], bp_bc[:])
                elif copy_eng is nc.scalar:
                    nc.scalar.copy(ot[:], ps[:, 0:C])
                else:
                    nc.vector.tensor_copy(ot[:], ps[:, 0:C])
                dma_eng.dma_start(out.ap()[it * 128:(it + 1) * 128, :],
                                  ot[:])

            def emit_proj(it, dma_eng, copy_eng=None):
                ps = psA.tile([128, 1024], F32, tag="big",
                              name=f"ops_{it}")
                _proj_mms(ps, it, range(KT))
                _proj_out(ps, it, dma_eng, copy_eng)

            def emit_proj_pair(it_a, it_b):
                # k5 (= pair 5) last and deferred past it_b's k0-4, so the
                # final pair's normalize chain is hidden behind PE work
                psa = psA.tile([128, 1024], F32, tag="big",
                               name=f"ops_{it_a}")
                _proj_mms(psa, it_a, range(KT - 1))
                psb = psA.tile([128, 1024], F32, tag="big",
                               name=f"ops_{it_b}")
                _proj_mms(psb, it_b, range(KT - 1))
                _proj_mms(psa, it_a, [KT - 1])
                _proj_mms(psb, it_b, [KT - 1])
                _proj_out(psa, it_a, nc.scalar, nc.scalar)
                _proj_out(psb, it_b, nc.sync, nc.scalar)

            # ---- chunk-major schedule ----
            qks = [None] * NPAIR
            qks[0] = emit_qkproj(0)
            emit_vproj()
            qks[1] = emit_qkproj(1)
            fin = None
            for hp in range(NPAIR):
                fin = emit_attn_chunk(hp, qks[hp], 0, fin)
                if hp + 2 < NPAIR:
                    qks[hp + 2] = emit_qkproj(hp + 2)
            for hp in range(NPAIR):
                fin = emit_attn_chunk(hp, qks[hp], 1, fin)
                if hp < 4:
                    emit_proj(hp, nc.sync)
            fin()
            emit_proj_pair(4, 5)
            emit_proj_pair(6, 7)

    nc.compile()
    return nc


def _get_program(bias_attn, bias_proj):
    key = (bias_attn, bias_proj)
    if key not in _cache:
        _cache[key] = _build_program(bias_attn, bias_proj)
    return _cache[key]


def _prep_inputs(x, w_attn, b_attn, w_proj, b_proj):
    x = np.asarray(x, dtype=np.float32)
    w_attn = np.asarray(w_attn, dtype=np.float32)
    b_attn = np.asarray(b_attn, dtype=np.float32)
    w_proj = np.asarray(w_proj, dtype=np.float32)
    b_proj = np.asarray(b_proj, dtype=np.float32)
    bias_attn = bool(np.any(b_attn))
    bias_proj = bool(np.any(b_proj))
    import ml_dtypes
    bf = ml_dtypes.bfloat16

    # wqkp[p, hp, k, j]: j<128 -> wq[128k+p, 128hp+j]; else wk[...]
    wq_r = w_attn[:, :C].reshape(KT, 128, NPAIR, 128)
    wk_r = w_attn[:, C:2 * C].reshape(KT, 128, NPAIR, 128)
    wqkp = np.ascontiguousarray(
        np.concatenate([wq_r, wk_r], axis=-1).transpose(1, 2, 0, 3)
    ).astype(bf)
    # pair-0 weights again, split in contiguous k-halves for the
    # startup-critical first DMAs
    wqk0h = np.ascontiguousarray(
        wqkp[:, 0].reshape(128, 2, 3 * 256).transpose(1, 0, 2))
    wvp = np.ascontiguousarray(
        w_attn[:, 2 * C:].reshape(KT, 128, C).transpose(1, 0, 2)).astype(bf)
    wvh = np.ascontiguousarray(
        wvp.reshape(128, 2, 3 * C).transpose(1, 0, 2))
    wpp = np.ascontiguousarray(
        w_proj.reshape(KT, 128, C).transpose(1, 0, 2)).astype(bf)

    in_maps = []
    for b in range(NCORES):
        xkp = np.ascontiguousarray(
            x[b].reshape(T, KT, 128).transpose(1, 2, 0)).astype(bf)
        m = {
            "xkp": xkp,
            "wqk0h": wqk0h,
            "wqkp": wqkp,
            "wvh": wvh,
            "wpp": wpp,
        }
        if bias_attn:
            m["bqk"] = np.ascontiguousarray(b_attn[:2 * C])
            m["bv"] = np.ascontiguousarray(b_attn[2 * C:])
        if bias_proj:
            m["bp"] = b_proj
        in_maps.append(m)
    return in_maps, bias_attn, bias_proj


def run(x, w_attn, b_attn, w_proj, b_proj, trace=False, tmpdir=None):
    in_maps, bias_attn, bias_proj = _prep_inputs(
        x, w_attn, b_attn, w_proj, b_proj)
    nc = _get_program(bias_attn, bias_proj)
    res = run_bass_kernel_spmd(nc, in_maps, list(range(NCORES)),
                               trace=trace, tmpdir=tmpdir)
    out = np.stack([res.results[i]["out"] for i in range(NCORES)], axis=0)
    return out.astype(np.float32), res


def kernel(x, w_attn, b_attn, w_proj, b_proj):
    out, _ = run(x, w_attn, b_attn, w_proj, b_proj)
    return out


# revision 42
# speedup vs baseline: 1.1165x; 1.1165x over previous
"""Causal self-attention Trainium2 Bass kernel.

Shapes (hardcoded): B=8, T=1024, C=768, NH=12, HS=64.
Sharding: data-parallel over batch — core b computes batch element b.

Per-core dataflow (all matmuls bf16 with fp32 PSUM accumulation):
  - Inputs are host-repacked so every DMA line is one contiguous
    segment per partition (128 descriptors per DMA): xTp [128,KT,T]
    split per k-tile, wqkp per head pair, wvp/wpp whole. Input DMAs are
    spread over both HWDGE rings (Sync + Scalar) so transfers overlap.
  - qkT [2,T] per pair = wqk tiles (stationary) x xT (moving); k-outer
    / nch-inner so each weight tile feeds 2 consecutive matmuls.
  - v token-major, assembled into v_aug [jt, head, 65] with a ones
    column so the PV matmul also emits softmax row-sums for free.
  - S^T blocks per head, COMPACTED: each j-tile's valid column range
    (512 - lo) is packed back-to-back in the PSUM tile so the exp
    activation covers only valid columns in a single op per group.
    Causality via block skipping plus an additive -1920 lower-
    triangular constant accumulated into diagonal blocks by a bf16
    matmul (id.T @ mtri); exp(0.125*(S-1920)) underflows to exact 0.
  - exp via ScalarE activation (scale=1/8) PSUM->SBUF into bf16 P^T,
    software-pipelined with PV groups as before.
  - y^T [65, i] = v_aug.T x P^T accumulated in PSUM; row 64 is the
    softmax denominator. Normalize: srow copy + reciprocal (DVE),
    gpsimd partition_broadcast, DVE multiply straight out of PSUM
    into bf16 yT (no intermediate copy).
  - Chunk-major schedule: all pairs' chunk 0 first (with pair hp+2's
    QK projection interleaved so its ScalarE copies stay ahead of the
    exp queue), then all pairs' chunk 1 with output-projection tiles
    it0..3 interleaved (their yT columns completed with chunk 0), then
    it4..7 at the end. Each out tile [128, 768] DMAs independently so
    the tail drain is one tile deep.
"""

import numpy as np

import concourse.bass as bass
import concourse.mybir as mybir
import concourse.tile as tile
from concourse import bacc
from concourse.bass_utils import run_bass_kernel_spmd

B, T, C = 8, 1024, 768
NH, HS = 12, 64
NCORES = 8
KT = C // 128            # 6 contraction tiles
NPAIR = NH // 2          # 6 head pairs; head-pair hp covers heads 2hp, 2hp+1
F32 = mybir.dt.float32
BF16 = mybir.dt.bfloat16
F8 = mybir.dt.float8e4

# compacted causal layout per chunk c: j-tile widths / offsets.
# Block placement keeps every matmul PSUM write within one 512-f32 bank.
_CHUNK = {}
for _c in (0, 1):
    _njt = 4 * (_c + 1)
    _w = [512 - max(0, (_jt - 4 * _c) * 128) for _jt in range(_njt)]
    _lo = [512 - _wi for _wi in _w]
    _base = [sum(_w[:_jt]) for _jt in range(_njt)]
    _CHUNK[_c] = (_njt, _w, _lo, _base, sum(_w))

_cache = {}


def _build_program(bias_attn: bool, bias_proj: bool):
    nc = bacc.Bacc("TRN2", target_bir_lowering=False, debug=False,
                   num_devices=NCORES)

    xh = nc.dram_tensor("xh", [2, 128, 3 * T], BF16, kind="ExternalInput")
    x8h = nc.dram_tensor("x8h", [2, 128, 3 * T], F8, kind="ExternalInput")
    # qk weights pre-interleaved for DoubleRowSwInterleave:
    # [p, pair, k-pair, part, 256] with (A,B) column pairs reversed
    wqk8p = nc.dram_tensor("wqk8p", [128, NPAIR, KT // 2, 2, 256], F8,
                           kind="ExternalInput")
    wvh = nc.dram_tensor("wvh", [2, 128, 3 * C], BF16, kind="ExternalInput")
    wpp = nc.dram_tensor("wpp", [128, KT, C], BF16, kind="ExternalInput")
    if bias_attn:
        bqk_d = nc.dram_tensor("bqk", [2 * C], F32, kind="ExternalInput")
        bv_d = nc.dram_tensor("bv", [C], F32, kind="ExternalInput")
    if bias_proj:
        bp_d = nc.dram_tensor("bp", [C], F32, kind="ExternalInput")
    # bf16 output (host casts back to f32): halves the writeback DMA
    out = nc.dram_tensor("out", [T, C], BF16, kind="ExternalOutput")

    # Constants merged into one inline tensor (one DMA): [id|mtri|mtri].
    # mtri is the additive causal mask for diagonal S^T blocks, applied
    # as a bf16 accumulation matmul (identity.T @ [mtri|mtri] adds mtri
    # to both diagonal blocks of a group in one instruction): -1920
    # pre-scale = -240 post-scale -> exp underflows to exactly 0.
    import ml_dtypes
    _mtri = np.tril(np.ones((128, 128), dtype=np.float32), k=-1) * -1920.0
    const_np = np.concatenate(
        [np.eye(128, dtype=np.float32), _mtri, _mtri],
        axis=1).astype(ml_dtypes.bfloat16)
    const_d = nc.inline_tensor(const_np, "idmtri")

    with tile.TileContext(nc) as tc:
        with (
            tc.tile_pool(name="xpool", bufs=1) as xpool,
            tc.tile_pool(name="cpool", bufs=1) as cpool,
            tc.tile_pool(name="wvpool", bufs=1) as wvpool,
            tc.tile_pool(name="vpool", bufs=1) as vpool,
            tc.tile_pool(name="wqkpool", bufs=1) as wqkpool,
            tc.tile_pool(name="qkpool", bufs=NPAIR) as qkpool,
            tc.tile_pool(name="ptpool", bufs=4) as ptpool,
            tc.tile_pool(name="ytpool", bufs=1) as ytpool,
            tc.tile_pool(name="wppool", bufs=1) as wppool,
            tc.tile_pool(name="opool", bufs=3) as opool,
            tc.tile_pool(name="smpool", bufs=4) as smpool,
            tc.tile_pool(name="psA", bufs=3, space="PSUM") as psA,
            tc.tile_pool(name="psB", bufs=2, space="PSUM") as psB,
        ):
            # ---- constants / bias staging (Scalar ring: keeps the Sync
            # ring free for the startup-critical wqk0/xT transfers) ----
            const_s = cpool.tile([128, 384], BF16, tag="idmtri")
            nc.scalar.dma_start(const_s[:], const_d.ap())
            id_s = const_s[:, 0:128]
            mtri2_s = const_s[:, 128:384]
            if bias_attn:
                bqk_s = cpool.tile([128, 12], F32, tag="bqk")
                nc.scalar.dma_start(
                    bqk_s[:], bqk_d.ap().rearrange("(m p) -> p m", p=128))
                bv_row = cpool.tile([1, C], F32, tag="bvrow")
                nc.scalar.dma_start(bv_row[:], bv_d.ap().rearrange("c -> 1 c"))
                bv_bc = cpool.tile([128, C], F32, tag="bvbc")
                nc.gpsimd.partition_broadcast(bv_bc[:], bv_row[:])
            if bias_proj:
                bp_row = cpool.tile([1, C], F32, tag="bprow")
                nc.scalar.dma_start(bp_row[:], bp_d.ap().rearrange("c -> 1 c"))
                bp_bc = cpool.tile([128, C], F32, tag="bpbc")
                nc.gpsimd.partition_broadcast(bp_bc[:], bp_row[:])

            ones3 = cpool.tile([128, NH, 1], F32, tag="ones3")
            nc.vector.memset(ones3[:], 1.0)

            # ---- input DMAs: few, fat-lined (contiguous per-partition
            # source rows), need-ordered across both HWDGE rings so HBM
            # bandwidth goes to whatever the PE needs next: fp8 x + pair-0
            # qk weights first (QK proj), then bf16 x + wv (V proj), then
            # later pairs and wp.
            wqk8_s = wqkpool.tile([128, NPAIR, KT // 2, 2, 256], F8,
                                  tag="wqk")
            x8_s = xpool.tile([128, KT, T], F8, tag="x8")
            xT_s = xpool.tile([128, KT, T], BF16, tag="xT")
            wv_s = wvpool.tile([128, KT, C], BF16, tag="wv")
            wp_s = wppool.tile([128, KT, C], BF16, tag="wp")

            def _half_dma(eng, dst3, src, h):
                eng.dma_start(dst3.rearrange("p k m -> p (k m)"),
                              src.ap().rearrange("h p m -> p h m")[:, h])

            nc.sync.dma_start(wqk8_s[:, 0], wqk8p.ap()[:, 0])
            _half_dma(nc.sync, x8_s[:, 0:3], x8h, 0)
            _half_dma(nc.scalar, x8_s[:, 3:6], x8h, 1)
            _half_dma(nc.sync, xT_s[:, 0:3], xh, 0)
            _half_dma(nc.scalar, xT_s[:, 3:6], xh, 1)
            _half_dma(nc.sync, wv_s[:, 0:3], wvh, 0)
            _half_dma(nc.scalar, wv_s[:, 3:6], wvh, 1)
            for hp in range(1, NPAIR):
                eng = nc.sync if hp % 2 == 1 else nc.scalar
                eng.dma_start(wqk8_s[:, hp], wqk8p.ap()[:, hp])
            nc.scalar.dma_start(wp_s[:], wpp.ap())

            def emit_qkproj(hp):
                qk_t = qkpool.tile([128, 2, T], BF16, tag="qk",
                                   name=f"qk_{hp}")
                for part in range(2):  # 0 = q m-tile hp, 1 = k m-tile hp
                    ps = psA.tile([128, 1024], F32, tag="big",
                                  name=f"qkps_{hp}_{part}")
                    # fp8 DoubleRow: two k-tiles contracted per pass
                    for kk in range(KT // 2):
                        for nch in range(2):
                            nc.tensor.matmul(
                                ps[:, nch * 512:(nch + 1) * 512],
                                wqk8_s[:, hp, kk, part].rearrange(
                                    "p (a m) -> p a m", m=128),
                                x8_s[:, 2 * kk:2 * kk + 2,
                                     nch * 512:(nch + 1) * 512],
                                start=(kk == 0), stop=(kk == KT // 2 - 1),
                                perf_mode=(
                                    mybir.MatmulPerfMode.DoubleRowSwInterleave),
                            )
                    # PSUM->SBUF on ScalarE: lands in the ACT queue ahead
                    # of downstream exps (chunk-0 phase has Scalar slack).
                    if bias_attn:
                        nc.scalar.add(qk_t[:, part, :], ps[:],
                                      bqk_s[:, part * 6 + hp:part * 6 + hp + 1])
                    else:
                        nc.scalar.copy(qk_t[:, part, :], ps[:])
                return qk_t

            # ---- V: token-major, assembled as v_aug[jt, head, 65] ----
            v_aug = vpool.tile([128, 8, NH, HS + 1], BF16, tag="vaug")

            def emit_vproj():
                for jt in range(8):
                    ps = psA.tile([128, 1024], F32, tag="big",
                                  name=f"vps_{jt}")
                    for k in range(KT):
                        for off, w in ((0, 512), (512, 256)):
                            nc.tensor.matmul(
                                ps[:, off:off + w],
                                xT_s[:, k, jt * 128:(jt + 1) * 128],
                                wv_s[:, k, off:off + w],
                                start=(k == 0), stop=(k == KT - 1),
                            )
                    dst = v_aug[:, jt, :, 0:HS]
                    src = ps[:, 0:C].rearrange("p (h d) -> p h d", d=HS)
                    if bias_attn:
                        nc.vector.tensor_add(
                            dst, src,
                            bv_bc[:].rearrange("p (h d) -> p h d", d=HS))
                    else:
                        nc.vector.tensor_copy(dst, src)
                    nc.vector.tensor_copy(v_aug[:, jt, :, HS:HS + 1],
                                          ones3[:])

            # ---- yT accumulator (written during attention) ----
            yT_s = ytpool.tile([128, KT, T], BF16, tag="yT")

            def emit_attn_chunk(hp, qk_t, c, pending):
                """Emit one (pair, chunk). The last PV group + normalize
                are NOT emitted here — they are returned as a closure and
                emitted inside the NEXT chunk (after its first exp), so
                the in-order PE queue never stalls on the chunk-boundary
                exp -> PV -> normalize chain."""
                njt, wj, lo, base, Wc = _CHUNK[c]
                ng = njt // 2
                pts = [ptpool.tile([128, Wc], BF16, tag="pt",
                                   name=f"pt_{hp}_{hl}_{c}")
                       for hl in range(2)]
                y_pss = [psB.tile([128, 512], F32, tag="y",
                                  name=f"yps_{hp}_{hl}_{c}")
                         for hl in range(2)]

                def emit_pv(g):
                    for u in range(2):
                        jt = 2 * g + u
                        for hl in range(2):
                            nc.tensor.matmul(
                                y_pss[hl][0:HS + 1, lo[jt]:512],
                                v_aug[:, jt, 2 * hp + hl, :],
                                pts[hl][:, base[jt]:base[jt] + wj[jt]],
                                start=(jt == 0),
                                stop=(jt == njt - 1),
                                skip_group_check=(jt > 0),
                            )

                # Software-pipelined emission: PV of group g-1 between
                # S^T groups so the in-order PE queue always has work
                # that does not wait on the ScalarE exp.
                for g in range(ng):
                    sts = [psA.tile([128, 1024], F32, tag="big",
                                    name=f"st_{hp}_{hl}_{c}_{g}")
                           for hl in range(2)]
                    gbase = base[2 * g]
                    gw = wj[2 * g] + wj[2 * g + 1]
                    for u in range(2):
                        jt = 2 * g + u
                        b_in = base[jt] - gbase
                        diag = jt >= 4 * c
                        for hl in range(2):
                            bp = 64 * hl
                            nc.tensor.matmul(
                                sts[hl][:, b_in:b_in + wj[jt]],
                                qk_t[bp:bp + 64, 1,
                                     jt * 128:(jt + 1) * 128],
                                qk_t[bp:bp + 64, 0,
                                     c * 512 + lo[jt]:(c + 1) * 512],
                                start=True, stop=not diag,
                            )
                        if diag:
                            # accumulate -1920 on the j>i triangle; the
                            # diag block is the first 128 compacted cols
                            for hl in range(2):
                                nc.tensor.matmul(
                                    sts[hl][:, b_in:b_in + 128],
                                    id_s, mtri2_s[:, 0:128],
                                    start=False, stop=True,
                                    skip_group_check=True,
                                )
                    for hl in range(2):
                        nc.scalar.activation(
                            pts[hl][:, gbase:gbase + gw],
                            sts[hl][:, 0:gw],
                            mybir.ActivationFunctionType.Exp,
                            scale=0.125,
                        )
                    if g == 0 and pending is not None:
                        pending()
                    if g >= 1:
                        emit_pv(g - 1)

                def finish():
                    emit_pv(ng - 1)
                    # normalize: yT = y / sums, multiplying straight out
                    # of PSUM; batched by op type so the in-order Vector
                    # queue is not blocked by the gpsimd broadcast.
                    # copy from PSUM (shifts partition 64 -> 0; HW
                    # gpsimd broadcast reads partition 0 only) and recip
                    # in place (custom-DVE needs base partition 0)
                    srows = []
                    for hl in range(2):
                        srow = smpool.tile([1, 512], F32, tag="srow",
                                           name=f"srow_{hp}_{hl}_{c}")
                        nc.vector.tensor_copy(srow[:],
                                              y_pss[hl][HS:HS + 1, :])
                        nc.vector.reciprocal_approx_fast(srow[:], srow[:])
                        srows.append(srow)
                    sbcs = []
                    for hl in range(2):
                        sbc = smpool.tile([64, 512], F32, tag="sbc",
                                          name=f"sbc_{hp}_{hl}_{c}")
                        nc.gpsimd.partition_broadcast(sbc[:], srows[hl][:])
                        sbcs.append(sbc)
                    for hl in range(2):
                        nc.vector.tensor_mul(
                            yT_s[64 * hl:64 * hl + 64, hp,
                                 c * 512:(c + 1) * 512],
                            y_pss[hl][0:HS, :], sbcs[hl][:])
                return finish

            # ---- projection tile: out[it] = yT.T x w_proj (+ b_proj) ----
            def _proj_mms(ps, it, ks):
                for k in ks:
                    for off, w in ((0, 512), (512, 256)):
                        nc.tensor.matmul(
                            ps[:, off:off + w],
                            yT_s[:, k, it * 128:(it + 1) * 128],
                            wp_s[:, k, off:off + w],
                            start=(k == 0), stop=(k == KT - 1),
                        )

            def _proj_out(ps, it, dma_eng, copy_eng):
                ot = opool.tile([128, C], BF16, tag="ot", name=f"ot_{it}")
                if bias_proj:
                    nc.vector.tensor_add(ot[:], ps[:, 0:C], bp_bc[:])
                elif copy_eng is nc.scalar:
                    nc.scalar.copy(ot[:], ps[:, 0:C])
                else:
                    nc.vector.tensor_copy(ot[:], ps[:, 0:C])
                dma_eng.dma_start(out.ap()[it * 128:(it + 1) * 128, :],
                                  ot[:])

            def emit_proj(it, dma_eng, copy_eng=None):
                ps = psA.tile([128, 1024], F32, tag="big",
                              name=f"ops_{it}")
                _proj_mms(ps, it, range(KT))
                _proj_out(ps, it, dma_eng, copy_eng)

            def emit_proj_pair(it_a, it_b):
                # k5 (= pair 5) last and deferred past it_b's k0-4, so the
                # final pair's normalize chain is hidden behind PE work
                psa = psA.tile([128, 1024], F32, tag="big",
                               name=f"ops_{it_a}")
                _proj_mms(psa, it_a, range(KT - 1))
                psb = psA.tile([128, 1024], F32, tag="big",
                               name=f"ops_{it_b}")
                _proj_mms(psb, it_b, range(KT - 1))
                _proj_mms(psa, it_a, [KT - 1])
                _proj_mms(psb, it_b, [KT - 1])
                _proj_out(psa, it_a, nc.scalar, nc.scalar)
                _proj_out(psb, it_b, nc.sync, nc.scalar)

            # ---- chunk-major schedule ----
            qks = [None] * NPAIR
            qks[0] = emit_qkproj(0)
            qks[1] = emit_qkproj(1)
            emit_vproj()
            fin = None
            for hp in range(NPAIR):
                fin = emit_attn_chunk(hp, qks[hp], 0, fin)
                if hp + 2 < NPAIR:
                    qks[hp + 2] = emit_qkproj(hp + 2)
            for hp in range(NPAIR):
                fin = emit_attn_chunk(hp, qks[hp], 1, fin)
                if hp < 4:
                    emit_proj(hp, nc.sync)
            fin()
            emit_proj_pair(4, 5)
            emit_proj_pair(6, 7)

    nc.compile()
    return nc


def _get_program(bias_attn, bias_proj):
    key = (bias_attn, bias_proj)
    if key not in _cache:
        _cache[key] = _build_program(bias_attn, bias_proj)
    return _cache[key]


def _prep_inputs(x, w_attn, b_attn, w_proj, b_proj):
    x = np.asarray(x, dtype=np.float32)
    w_attn = np.asarray(w_attn, dtype=np.float32)
    b_attn = np.asarray(b_attn, dtype=np.float32)
    w_proj = np.asarray(w_proj, dtype=np.float32)
    b_proj = np.asarray(b_proj, dtype=np.float32)
    bias_attn = bool(np.any(b_attn))
    bias_proj = bool(np.any(b_proj))
    import ml_dtypes
    bf = ml_dtypes.bfloat16

    f8 = ml_dtypes.float8_e4m3
    # qk weights for DoubleRowSwInterleave: for k-tile pair (A, B) =
    # (2kk, 2kk+1), each partition row holds [A127,B127,A126,B126,...,B0]
    # (A/B column pairs, columns reversed)
    wq_r = w_attn[:, :C].reshape(KT, 128, NPAIR, 128)
    wk_r = w_attn[:, C:2 * C].reshape(KT, 128, NPAIR, 128)
    wqk_all = np.concatenate([wq_r, wk_r], axis=-1)   # [k, p, hp, 256]
    w6 = wqk_all.reshape(KT // 2, 2, 128, NPAIR, 2, 128)
    il = np.stack([w6[:, 0, ..., ::-1], w6[:, 1, ..., ::-1]], axis=-1)
    # il: [kk, p, hp, part, 128, 2] -> [p, hp, kk, part, 256]
    wqk8p = np.ascontiguousarray(
        il.reshape(KT // 2, 128, NPAIR, 2, 256).transpose(1, 2, 0, 3, 4)
    ).astype(f8)
    wvp = np.ascontiguousarray(
        w_attn[:, 2 * C:].reshape(KT, 128, C).transpose(1, 0, 2)).astype(bf)
    wvh = np.ascontiguousarray(
        wvp.reshape(128, 2, 3 * C).transpose(1, 0, 2))
    wpp = np.ascontiguousarray(
        w_proj.reshape(KT, 128, C).transpose(1, 0, 2)).astype(bf)

    in_maps = []
    for b in range(NCORES):
        xTp = np.ascontiguousarray(
            x[b].reshape(T, KT, 128).transpose(2, 1, 0))
        xhh = np.ascontiguousarray(
            xTp.reshape(128, 2, 3 * T).transpose(1, 0, 2))
        m = {
            "xh": xhh.astype(bf),
            "x8h": xhh.astype(f8),
            "wqk8p": wqk8p,
            "wvh": wvh,
            "wpp": wpp,
        }
        if bias_attn:
            m["bqk"] = np.ascontiguousarray(b_attn[:2 * C])
            m["bv"] = np.ascontiguousarray(b_attn[2 * C:])
        if bias_proj:
            m["bp"] = b_proj
        in_maps.append(m)
    return in_maps, bias_attn, bias_proj


def run(x, w_attn, b_attn, w_proj, b_proj, trace=False, tmpdir=None):
    in_maps, bias_attn, bias_proj = _prep_inputs(
        x, w_attn, b_attn, w_proj, b_proj)
    nc = _get_program(bias_attn, bias_proj)
    res = run_bass_kernel_spmd(nc, in_maps, list(range(NCORES)),
                               trace=trace, tmpdir=tmpdir)
    out = np.stack([res.results[i]["out"] for i in range(NCORES)], axis=0)
    return out.astype(np.float32), res


def kernel(x, w_attn, b_attn, w_proj, b_proj):
    out, _ = run(x, w_attn, b_attn, w_proj, b_proj)
    return out
